# revision 1
# baseline (speedup 1.0000x reference)
"""GCN layer (GCNConv + BatchNorm1d + ReLU + residual) on 8 Trainium2 cores.

Strategy (dst-sharded):
  * Nodes are sharded by destination across the 8 cores (12500 nodes each).
  * Host preprocessing (index-only): append self-loops, sort edges by dst,
    bucket them into per-core / per-128-dst-node "windows", pad each window's
    edge list to whole 128-edge blocks so all 8 cores share one SPMD program.
  * Device, per core:
      - deg from CSR indptr diff, dinv = 1/sqrt(deg)
      - h = (x_loc @ W.T) * dinv[node]   (PE matmul, xT stationary)
      - AllGather h -> full h table in HBM
      - main loop: indirect-DMA gather of h[src] rows (128 edges/block),
        build one-hot selection matrix S[e,d] = (dst_rel[e]==d) on DVE,
        PE matmul  psum[feat,dst] += gathered^T @ S  accumulated per window,
        evict with dinv[dst] scaling fused with BN-stat accumulation.
      - AllReduce per-feature sum/sumsq -> BN affine -> ReLU -> +x -> out.
Output returned transposed per core; host concatenates and trims padding.
"""

import math
from contextlib import ExitStack

import numpy as np

P = 128
D = 128
BN_EPS = 1e-5

N_FULL = 100000
N_CORES = 8
GQ = 1  # 128-edge blocks per indirect-DMA gather call (HW: 1 index/partition)


# ---------------------------------------------------------------------------
# Host-side index preprocessing (sharding layout only; all arithmetic on the
# tensor data happens on device).
# ---------------------------------------------------------------------------
def make_plan(edge_index: np.ndarray, n: int, n_cores: int, gq: int = GQ):
    assert n % n_cores == 0
    n_loc = n // n_cores
    n_win = math.ceil(n_loc / P)
    n_pad = n_win * P

    src = np.asarray(edge_index[0], dtype=np.int64)
    dst = np.asarray(edge_index[1], dtype=np.int64)
    loop = np.arange(n, dtype=np.int64)
    src = np.concatenate([src, loop])
    dst = np.concatenate([dst, loop])

    order = np.argsort(dst, kind="stable")
    ssrc = src[order]
    sdst = dst[order]

    # per-(core, window) edge counts -> shared SPMD block structure
    cnt = np.zeros((n_cores, n_win), dtype=np.int64)
    seg_bounds = np.searchsorted(sdst, np.arange(n_cores + 1) * n_loc)
    core_lo = []
    for k in range(n_cores):
        lo, hi = seg_bounds[k], seg_bounds[k + 1]
        local = sdst[lo:hi] - k * n_loc
        cnt[k] = np.bincount(local // P, minlength=n_win)
        core_lo.append((lo, hi))

    nblk = np.maximum(1, -(-cnt.max(axis=0) // P))  # ceil, >=1
    t_blocks = int(nblk.sum())
    extra = (-t_blocks) % gq
    nblk[-1] += extra  # dummy blocks appended to the last window
    t_blocks += extra
    blk_start = np.concatenate([[0], np.cumsum(nblk)]).astype(np.int64)

    block_to_win = np.repeat(np.arange(n_win), nblk)

    src_arr = np.zeros((n_cores, P, t_blocks), dtype=np.int32)
    drel_arr = np.full((n_cores, P, t_blocks), -1.0, dtype=np.float32)
    indptr_arr = np.zeros((n_cores, n_pad + 1), dtype=np.int32)

    for k in range(n_cores):
        lo, hi = core_lo[k]
        local = sdst[lo:hi] - k * n_loc
        srck = ssrc[lo:hi]
        counts = np.bincount(local, minlength=n_pad)
        indptr_arr[k, 1:] = np.cumsum(counts).astype(np.int32)

        wstart = np.searchsorted(local // P, np.arange(n_win + 1))
        for w in range(n_win):
            a, b = wstart[w], wstart[w + 1]
            s_w = srck[a:b]
            d_w = local[a:b] - w * P
            # sort by src for HBM locality during the gather
            o2 = np.argsort(s_w, kind="stable")
            s_w = s_w[o2]
            d_w = d_w[o2]
            # map src to padded-global row index in the all-gathered table
            ks = s_w // n_loc
            s_pad = (ks * n_pad + (s_w - ks * n_loc)).astype(np.int32)
            m = b - a
            j = np.arange(m)
            bidx = blk_start[w] + j // P
            lane = j % P
            src_arr[k, lane, bidx] = s_pad
            drel_arr[k, lane, bidx] = d_w.astype(np.float32)

    return dict(
        n=n,
        n_cores=n_cores,
        n_loc=n_loc,
        n_win=n_win,
        n_pad=n_pad,
        gq=gq,
        t_blocks=t_blocks,
        nblk=nblk,
        blk_start=blk_start,
        block_to_win=block_to_win,
        src_arr=src_arr,
        drel_arr=drel_arr,
        indptr_arr=indptr_arr,
    )


# ---------------------------------------------------------------------------
# Device program
# ---------------------------------------------------------------------------
def build_nc(plan, stage=99):
    import concourse.bacc as bacc
    import concourse.bass as bass
    import concourse.mybir as mybir
    import concourse.tile as tile
    from concourse.masks import make_identity

    f32 = mybir.dt.float32
    bf16 = mybir.dt.bfloat16
    i32 = mybir.dt.int32
    AF = mybir.ActivationFunctionType
    OP = mybir.AluOpType

    n = plan["n"]
    n_cores = plan["n_cores"]
    n_win = plan["n_win"]
    n_pad = plan["n_pad"]
    gq = plan["gq"]
    t_blocks = plan["t_blocks"]
    nblk = plan["nblk"]
    b2w = plan["block_to_win"]
    blk_start = plan["blk_start"]

    nc = bacc.Bacc(
        "TRN2", target_bir_lowering=False, debug=False, num_devices=n_cores
    )

    xT = nc.dram_tensor("xT", [P, n_pad], f32, kind="ExternalInput")
    wt = nc.dram_tensor("wt", [P, P], f32, kind="ExternalInput")
    indptr = nc.dram_tensor("indptr", [n_pad + 1], i32, kind="ExternalInput")
    srci = nc.dram_tensor("srci", [P, t_blocks], i32, kind="ExternalInput")
    drel = nc.dram_tensor("drel", [P, t_blocks], bf16, kind="ExternalInput")
    iota_in = nc.dram_tensor("iota", [P, P], bf16, kind="ExternalInput")
    gam = nc.dram_tensor("gam", [P, 1], f32, kind="ExternalInput")
    bet = nc.dram_tensor("bet", [P, 1], f32, kind="ExternalInput")
    out_d = nc.dram_tensor("out", [P, n_pad], f32, kind="ExternalOutput")

    rg = [list(range(n_cores))]

    with tile.TileContext(nc) as tc, ExitStack() as ctx:
        const = ctx.enter_context(tc.tile_pool(name="const", bufs=1))
        work = ctx.enter_context(tc.tile_pool(name="work", bufs=3))
        pre_ps = ctx.enter_context(tc.tile_pool(name="pre_ps", bufs=2, space="PSUM"))
        win_ps = ctx.enter_context(tc.tile_pool(name="win_ps", bufs=2, space="PSUM"))
        brd_ps = ctx.enter_context(tc.tile_pool(name="brd_ps", bufs=2, space="PSUM"))
        dram = ctx.enter_context(tc.tile_pool(name="dram", bufs=1, space="DRAM"))

        # ---- constants / inputs resident in SBUF
        xT_sb = const.tile([P, n_pad], f32)
        nc.sync.dma_start(out=xT_sb[:], in_=xT[:, :])
        wt_sb = const.tile([P, P], f32)
        nc.sync.dma_start(out=wt_sb[:], in_=wt[:, :])
        iota_sb = const.tile([P, P], bf16)
        nc.sync.dma_start(out=iota_sb[:], in_=iota_in[:, :])
        gam_sb = const.tile([P, 1], f32)
        nc.sync.dma_start(out=gam_sb[:], in_=gam[:, :])
        bet_sb = const.tile([P, 1], f32)
        nc.sync.dma_start(out=bet_sb[:], in_=bet[:, :])
        ones_full = const.tile([P, P], f32)
        nc.vector.memset(ones_full[:], 1.0)
        ident_sb = const.tile([P, P], f32)
        make_identity(nc, ident_sb[:])

        src_sb = const.tile([P, t_blocks], i32)
        nc.sync.dma_start(out=src_sb[:], in_=srci[:, :])
        drel_sb = const.tile([P, t_blocks], bf16)
        nc.sync.dma_start(out=drel_sb[:], in_=drel[:, :])

        # ---- degree -> dinv, in two layouts
        # column layout [node_in_window(part), window]
        ipA_c = const.tile([P, n_win], i32)
        nc.sync.dma_start(
            out=ipA_c[:], in_=indptr[0:n_pad].rearrange("(w p) -> p w", p=P)
        )
        ipB_c = const.tile([P, n_win], i32)
        nc.sync.dma_start(
            out=ipB_c[:], in_=indptr[1 : n_pad + 1].rearrange("(w p) -> p w", p=P)
        )
        deg_ci = const.tile([P, n_win], i32)
        nc.vector.tensor_sub(deg_ci[:], ipB_c[:], ipA_c[:])
        dinv_c = const.tile([P, n_win], f32)
        nc.vector.tensor_copy(dinv_c[:], deg_ci[:])
        nc.vector.tensor_scalar_max(dinv_c[:], dinv_c[:], 1.0)
        nc.scalar.sqrt(dinv_c[:], dinv_c[:])
        nc.vector.reciprocal(dinv_c[:], dinv_c[:])

        if stage <= 1:  # debug: dinv only
            nc.sync.dma_start(out=out_d[:, 0:n_win], in_=dinv_c[:])

        # ---- preamble: hs = (x @ W.T) * dinv  -> hs_loc, then AllGather
        hs_loc = dram.tile([n_pad, 2 * P], bf16)
        hs_full = dram.tile([n_pad * n_cores, 2 * P], bf16)
        for w in range(n_win if stage >= 2 else 0):
            ph = pre_ps.tile([P, P], f32, tag="ph")
            nc.tensor.matmul(
                out=ph[:],
                lhsT=xT_sb[:, w * P : (w + 1) * P],
                rhs=wt_sb[:],
                start=True,
                stop=True,
            )
            # split hs into bf16 hi + lo so the edge matmuls can run in
            # bf16 while keeping ~fp32 end-to-end precision
            hs_f = work.tile([P, P], f32, tag="hs_f")
            nc.scalar.activation(
                out=hs_f[:], in_=ph[:], func=AF.Copy, scale=dinv_c[:, w : w + 1]
            )
            hs_t = work.tile([P, 2 * P], bf16, tag="hs_t")
            nc.vector.tensor_copy(hs_t[:, 0:P], hs_f[:])
            hi_f = work.tile([P, P], f32, tag="hi_f")
            nc.vector.tensor_copy(hi_f[:], hs_t[:, 0:P])
            nc.vector.tensor_sub(hs_t[:, P : 2 * P], hs_f[:], hi_f[:])
            nc.sync.dma_start(out=hs_loc[w * P : (w + 1) * P, :], in_=hs_t[:])

        if stage >= 2:
            nc.gpsimd.collective_compute(
                "AllGather",
                mybir.AluOpType.bypass,
                replica_groups=rg,
                ins=[hs_loc[:].opt()],
                outs=[hs_full[:].opt()],
            )

        if stage == 2:  # debug: preamble + AG only
            tdbg = work.tile([P, P], f32, tag="tdbg")
            nc.sync.dma_start(out=tdbg[:], in_=hs_full[0:P, :])
            nc.sync.dma_start(out=out_d[:, 0:P], in_=tdbg[:])

        # ---- main loop: gather + selection-matmul per window
        agg = const.tile([P, n_win * P], f32)
        sum_c = const.tile([P, n_win], f32)
        sq_c = const.tile([P, n_win], f32)
        trash = const.tile([P, 1], f32)

        last_blk = blk_start[1:] - 1  # last block index of each window
        cur_tile = None
        for b in range(t_blocks if stage >= 3 else 0):
            # HW indirect DMA honors exactly one index per partition, so
            # each 128-edge block is one gather call (Q7 emission bound).
            gt = work.tile([P, 2 * P], bf16, tag="gt", bufs=8)
            nc.gpsimd.indirect_dma_start(
                out=gt[:],
                out_offset=None,
                in_=hs_full[:, :],
                in_offset=bass.IndirectOffsetOnAxis(
                    ap=src_sb[:, b : b + 1], axis=0
                ),
            )
            s2 = work.tile([P, P], bf16, tag="s2", bufs=4)
            nc.vector.tensor_tensor(
                out=s2[:],
                in0=drel_sb[:, b : b + 1].to_broadcast([P, P]),
                in1=iota_sb[:],
                op=OP.is_equal,
            )
            if True:
                w = int(b2w[b])
                if b == blk_start[w]:
                    cur_tile = win_ps.tile([P, P], f32, tag="win")
                nc.tensor.matmul(
                    out=cur_tile[:],
                    lhsT=gt[:, 0:P],
                    rhs=s2[:],
                    start=(b == blk_start[w]),
                    stop=False,
                )
                nc.tensor.matmul(
                    out=cur_tile[:],
                    lhsT=gt[:, P : 2 * P],
                    rhs=s2[:],
                    start=False,
                    stop=(b == last_blk[w]),
                )
                if b == last_blk[w] and stage <= 3:
                    # debug evict: plain copy, no stats
                    nc.scalar.activation(
                        out=agg[:, w * P : (w + 1) * P], in_=cur_tile[:], func=AF.Copy
                    )
                if b == last_blk[w] and stage >= 4:
                    # evict: scale by dinv[dst] and accumulate BN stats.
                    # bp[f, d] = dinv[d], built as ones.T @ diag(dinv_w)
                    diag_t = work.tile([P, P], f32, tag="diag")
                    nc.vector.tensor_scalar_mul(
                        diag_t[:], ident_sb[:], dinv_c[:, w : w + 1]
                    )
                    bp = brd_ps.tile([P, P], f32, tag="brd")
                    nc.tensor.matmul(
                        out=bp[:],
                        lhsT=ones_full[:],
                        rhs=diag_t[:],
                        start=True,
                        stop=True,
                    )
                    db = work.tile([P, P], f32, tag="db")
                    nc.scalar.activation(out=db[:], in_=bp[:], func=AF.Copy)
                    a_sl = agg[:, w * P : (w + 1) * P]
                    nc.vector.tensor_mul(a_sl, cur_tile[:], db[:])
                    nc.vector.tensor_reduce(
                        out=sum_c[:, w : w + 1],
                        in_=a_sl,
                        axis=mybir.AxisListType.X,
                        op=OP.add,
                    )
                    sqt = work.tile([P, P], f32, tag="sqt")
                    nc.scalar.activation(
                        out=sqt[:],
                        in_=a_sl,
                        func=AF.Square,
                        accum_out=sq_c[:, w : w + 1],
                    )

        if stage in (3, 4):  # debug: dump agg
            nc.sync.dma_start(out=out_d[:, :], in_=agg[:])

        # ---- BN statistics all-reduce
        stot = const.tile([P, 2], f32)
        if stage >= 5:
            nc.vector.tensor_reduce(
            out=stot[:, 0:1], in_=sum_c[:], axis=mybir.AxisListType.X, op=OP.add
        )
            nc.vector.tensor_reduce(
                out=stot[:, 1:2], in_=sq_c[:], axis=mybir.AxisListType.X, op=OP.add
            )
            stats_l = dram.tile([P, 2], f32)
            stats_g = dram.tile([P, 2], f32)
            nc.sync.dma_start(out=stats_l[:, :], in_=stot[:])
            nc.gpsimd.collective_compute(
                "AllReduce",
                mybir.AluOpType.add,
                replica_groups=rg,
                ins=[stats_l[:].opt()],
                outs=[stats_g[:].opt()],
            )
            sg = const.tile([P, 2], f32)
            nc.sync.dma_start(out=sg[:], in_=stats_g[:, :])

            # ---- BN affine params: s = gamma/std, t = beta - mean*s
            mean = const.tile([P, 1], f32)
            nc.vector.tensor_scalar_mul(mean[:], sg[:, 0:1], 1.0 / n)
            var = const.tile([P, 1], f32)
            nc.vector.tensor_scalar_mul(var[:], sg[:, 1:2], 1.0 / n)
            msq = const.tile([P, 1], f32)
            nc.vector.tensor_mul(msq[:], mean[:], mean[:])
            nc.vector.tensor_sub(var[:], var[:], msq[:])
            nc.vector.tensor_scalar_add(var[:], var[:], BN_EPS)
            nc.scalar.sqrt(var[:], var[:])
            s_t = const.tile([P, 1], f32)
            nc.vector.reciprocal(s_t[:], var[:])
            nc.vector.tensor_mul(s_t[:], gam_sb[:], s_t[:])
            t_t = const.tile([P, 1], f32)
            nc.vector.tensor_mul(t_t[:], mean[:], s_t[:])
            nc.vector.tensor_sub(t_t[:], bet_sb[:], t_t[:])

            # ---- epilogue: out = relu(agg*s + t) + x
            for w in range(n_win):
                y = work.tile([P, P], f32, tag="y")
                nc.scalar.activation(
                    out=y[:],
                    in_=agg[:, w * P : (w + 1) * P],
                    func=AF.Relu,
                    scale=s_t[:],
                    bias=t_t[:],
                )
                y2 = work.tile([P, P], f32, tag="y2")
                nc.vector.tensor_add(y2[:], y[:], xT_sb[:, w * P : (w + 1) * P])
                nc.sync.dma_start(out=out_d[:, w * P : (w + 1) * P], in_=y2[:])

    nc.compile()
    return nc


# ---------------------------------------------------------------------------
# Host wrapper
# ---------------------------------------------------------------------------
def _in_maps(plan, x, W, gamma, beta):
    n_cores = plan["n_cores"]
    n_loc = plan["n_loc"]
    n_pad = plan["n_pad"]
    import ml_dtypes

    x = np.asarray(x, dtype=np.float32)
    wt = np.ascontiguousarray(np.asarray(W, dtype=np.float32).T)
    iota = np.tile(np.arange(P, dtype=ml_dtypes.bfloat16), (P, 1))
    gam = np.asarray(gamma, dtype=np.float32).reshape(P, 1)
    bet = np.asarray(beta, dtype=np.float32).reshape(P, 1)
    maps = []
    for k in range(n_cores):
        xk = x[k * n_loc : (k + 1) * n_loc]
        xkT = np.zeros((P, n_pad), dtype=np.float32)
        xkT[:, :n_loc] = xk.T
        maps.append(
            dict(
                xT=xkT,
                wt=wt,
                indptr=plan["indptr_arr"][k],
                srci=np.ascontiguousarray(plan["src_arr"][k]),
                drel=np.ascontiguousarray(
                    plan["drel_arr"][k].astype(ml_dtypes.bfloat16)
                ),
                iota=iota,
                gam=gam,
                bet=bet,
            )
        )
    return maps


def run(x, edge_index, W, b, gamma, beta, n=N_FULL, n_cores=N_CORES, trace=False):
    from concourse.bass_utils import run_bass_kernel_spmd

    plan = make_plan(np.asarray(edge_index), n, n_cores)
    nc = build_nc(plan)
    maps = _in_maps(plan, x, W, gamma, beta)
    res = run_bass_kernel_spmd(nc, maps, core_ids=list(range(n_cores)), trace=trace)
    n_loc = plan["n_loc"]
    out = np.concatenate(
        [res.results[k]["out"].T[:n_loc] for k in range(n_cores)], axis=0
    )
    return out, res


def kernel(x, edge_index, W, b, gamma, beta):
    out, _ = run(x, edge_index, W, b, gamma, beta)
    return out

